# revision 6
# baseline (speedup 1.0000x reference)
"""Trainium2 Bass kernel for nn_NodePreTrans (e3nn tensor product + linear).

Data-parallel over nodes: 50000 rows sharded 8 ways (6250/core, padded to
6272).  Channel-major on-device layout: all matmuls are (weights stationary)
[K,128] x [K,Z] with Z up to 512 nodes in the moving/free dimension.
"""

import sys

sys.path.insert(0, "/opt/trn_rl_repo")

import numpy as np

import concourse.bacc as bacc
import concourse.bass as bass
import concourse.mybir as mybir
import concourse.tile as tile
from concourse.bass_utils import run_bass_kernel_spmd

N_NODES = 50000
N_CORES = 8
NS = N_NODES // N_CORES          # 6250 real nodes per core
NSH = 6272                       # padded (12*512 + 128)
MUL_S = 256
MUL_V = 128

C_000 = 1.0 / np.sqrt(256.0)
C_011 = 1.0 / np.sqrt(128.0)
C_101 = 1.0 / np.sqrt(256.0)
C_110 = 1.0 / np.sqrt(384.0)
C_111 = 1.0 / 16.0

F32 = mybir.dt.float32
F32R = mybir.dt.float32r

Z_BLOCKS = [(i * 512, 512) for i in range(12)] + [(6144, 128)]

_CACHE = {}


def _build_program(variant="full"):
    nc = bacc.Bacc("TRN2", target_bir_lowering=False, debug=False,
                   num_devices=N_CORES)

    xT_d = nc.dram_tensor("xT", [640, NSH], F32, kind="ExternalInput").ap()
    wt000_d = nc.dram_tensor("wt000", [256, 256], F32, kind="ExternalInput").ap()
    wt011_d = nc.dram_tensor("wt011", [128, 256], F32, kind="ExternalInput").ap()
    wt101_d = nc.dram_tensor("wt101", [256, 128], F32, kind="ExternalInput").ap()
    wt110_d = nc.dram_tensor("wt110", [128, 128], F32, kind="ExternalInput").ap()
    wt111_d = nc.dram_tensor("wt111", [128, 128], F32, kind="ExternalInput").ap()
    l0e_d = nc.dram_tensor("l0e", [384, 256], F32, kind="ExternalInput").ap()
    l1o_d = nc.dram_tensor("l1o", [384, 128], F32, kind="ExternalInput").ap()
    l1e_d = nc.dram_tensor("l1e", [128, 128], F32, kind="ExternalInput").ap()
    outT_d = nc.dram_tensor("outT", [1024, NSH], F32, kind="ExternalOutput").ap()

    with tile.TileContext(nc) as tc:
        _emit(tc, nc, xT_d, wt000_d, wt011_d, wt101_d, wt110_d, wt111_d,
              l0e_d, l1o_d, l1e_d, outT_d, variant=variant)

    nc.compile()
    return nc


def _emit(tc, nc, xT_d, wt000_d, wt011_d, wt101_d, wt110_d, wt111_d,
          l0e_d, l1o_d, l1e_d, outT_d, variant="full"):
    with (
        tc.tile_pool(name="wpool", bufs=1) as wpool,
        tc.tile_pool(name="xin", bufs=2) as xin,
        tc.tile_pool(name="gat", bufs=2) as gat,
        tc.tile_pool(name="tmp", bufs=2) as tmp,
        tc.tile_pool(name="oev", bufs=2) as oev,
        tc.tile_pool(name="ps1", bufs=1, space="PSUM") as ps1,
        tc.tile_pool(name="ps2", bufs=1, space="PSUM") as ps2,
    ):
        # ---- resident weights -------------------------------------------
        def wtile(name, dram_ap, rows, cols):
            t = wpool.tile([128, cols], F32, name=name)
            nc.sync.dma_start(t[:, :], dram_ap[rows:rows + 128, :])
            return t

        w000 = [wtile(f"w000_{k}", wt000_d, 128 * k, 256) for k in range(2)]
        w011 = wtile("w011", wt011_d, 0, 256)
        w101 = [wtile(f"w101_{k}", wt101_d, 128 * k, 128) for k in range(2)]
        w110 = wtile("w110", wt110_d, 0, 128)
        w111 = wtile("w111", wt111_d, 0, 128)
        L0e = [wtile(f"l0e_{k}", l0e_d, 128 * k, 256) for k in range(3)]
        L1o = [wtile(f"l1o_{k}", l1o_d, 128 * k, 128) for k in range(3)]
        L1e = wtile("l1e", l1e_d, 0, 128)

        for z0, Z in Z_BLOCKS:
            # ---- load x tiles (channel-major) ---------------------------
            s = []
            for m in range(2):
                t = xin.tile([128, 512], F32, name=f"s{m}")
                nc.sync.dma_start(t[:, :Z], xT_d[128 * m:128 * (m + 1),
                                                 z0:z0 + Z])
                s.append(t)
            v = []
            for j in range(3):
                t = xin.tile([128, 512], F32, name=f"v{j}")
                nc.sync.dma_start(t[:, :Z], xT_d[256 + 128 * j:384 + 128 * j,
                                                 z0:z0 + Z])
                v.append(t)

            def ps_tile():
                return ps1.tile([128, 512], F32, name="s1r", bufs=5)

            if variant == "dma":
                for i, t in enumerate(s + v):
                    nc.sync.dma_start(outT_d[128 * i:128 * (i + 1),
                                             z0:z0 + Z], t[:, :Z])
                continue

            if variant == "mm":
                idx = 0
                for (w, rr) in [(w000[0], s[0]), (w000[1], s[1]),
                                (w011, v[0]), (w011, v[1]), (w011, v[2]),
                                (w101[0], s[0]), (w101[1], s[1]),
                                (w110, v[0]), (w110, v[1]), (w110, v[2]),
                                (w111, v[0]), (w111, v[1]), (w111, v[2])]:
                    a = ps_tile()
                    nc.tensor.matmul(a[:, :Z], w[:, :128], rr[:, :Z],
                                     start=True, stop=True)
                    ev = oev.tile([128, 512], F32, name=f"mmev{idx % 4}")
                    nc.scalar.copy(ev[:, :Z], a[:, :Z])
                    nc.sync.dma_start(outT_d[128 * (idx % 8):
                                             128 * (idx % 8) + 128,
                                             z0:z0 + Z], ev[:, :Z])
                    idx += 1
                continue

            # ---- path 1: p1 = s * (w00.T @ s) --------------------------
            p1 = []
            for m in range(2):
                a = ps_tile()
                nc.tensor.matmul(a[:, :Z], w000[0][:, 128 * m:128 * (m + 1)],
                                 s[0][:, :Z], start=True, stop=False)
                nc.tensor.matmul(a[:, :Z], w000[1][:, 128 * m:128 * (m + 1)],
                                 s[1][:, :Z], start=False, stop=True)
                p = gat.tile([128, 512], F32, name=f"p1_{m}")
                nc.vector.tensor_mul(p[:, :Z], s[m][:, :Z], a[:, :Z])
                p1.append(p)

            # ---- path 2: p2_j = s * (w01.T @ v_j) ----------------------
            p2 = []
            for j in range(3):
                pj = []
                for m in range(2):
                    b = ps_tile()
                    nc.tensor.matmul(b[:, :Z], w011[:, 128 * m:128 * (m + 1)],
                                     v[j][:, :Z], start=True, stop=True)
                    p = gat.tile([128, 512], F32, name=f"p2_{j}_{m}")
                    nc.vector.tensor_mul(p[:, :Z], s[m][:, :Z], b[:, :Z])
                    pj.append(p)
                p2.append(pj)

            # ---- path 3: p3_j = v_j * (w10.T @ s) ----------------------
            c = ps_tile()
            nc.tensor.matmul(c[:, :Z], w101[0][:, :], s[0][:, :Z],
                             start=True, stop=False)
            nc.tensor.matmul(c[:, :Z], w101[1][:, :], s[1][:, :Z],
                             start=False, stop=True)
            p3 = []
            for j in range(3):
                p = gat.tile([128, 512], F32, name=f"p3_{j}")
                nc.vector.tensor_mul(p[:, :Z], v[j][:, :Z], c[:, :Z])
                p3.append(p)

            # ---- path 4: p4 = sum_j v_j * (w110.T @ v_j) ---------------
            p4 = gat.tile([128, 512], F32, name="p4")
            for j in range(3):
                d = ps_tile()
                nc.tensor.matmul(d[:, :Z], w110[:, :], v[j][:, :Z],
                                 start=True, stop=True)
                if j == 0:
                    nc.vector.tensor_mul(p4[:, :Z], v[0][:, :Z], d[:, :Z])
                else:
                    t4 = tmp.tile([128, 512], F32, name="t4")
                    nc.vector.tensor_mul(t4[:, :Z], v[j][:, :Z], d[:, :Z])
                    nc.vector.tensor_add(p4[:, :Z], p4[:, :Z], t4[:, :Z])

            # ---- path 5: p5_k = v_i*E_j - v_j*E_i, (i,j)=(k+1,k+2)%3 ---
            E = []
            for j in range(3):
                e = ps_tile()
                nc.tensor.matmul(e[:, :Z], w111[:, :], v[j][:, :Z],
                                 start=True, stop=True)
                E.append(e)
            p5 = []
            for k in range(3):
                i, j = (k + 1) % 3, (k + 2) % 3
                ta = tmp.tile([128, 512], F32, name="t5a")
                tb = tmp.tile([128, 512], F32, name="t5b")
                nc.vector.tensor_mul(ta[:, :Z], v[i][:, :Z], E[j][:, :Z])
                nc.vector.tensor_mul(tb[:, :Z], v[j][:, :Z], E[i][:, :Z])
                p = gat.tile([128, 512], F32, name=f"p5_{k}")
                nc.vector.tensor_sub(p[:, :Z], ta[:, :Z], tb[:, :Z])
                p5.append(p)

            if variant == "gat":
                outs8 = [p1[0], p1[1], p2[0][0], p2[0][1], p3[0], p4,
                         p5[0], p5[1]]
                for i, t in enumerate(outs8):
                    nc.sync.dma_start(outT_d[128 * i:128 * (i + 1),
                                             z0:z0 + Z], t[:, :Z])
                continue

            # ---- stage 2 linears + evacuate + store --------------------
            def emit_out(name, row0, chunks):
                o = ps2.tile([128, 512], F32, name="s2o", bufs=3)
                n = len(chunks)
                for ci, (lw, rhs) in enumerate(chunks):
                    nc.tensor.matmul(o[:, :Z], lw, rhs[:, :Z],
                                     start=(ci == 0), stop=(ci == n - 1))
                ev = oev.tile([128, 512], F32, name=name)
                nc.scalar.copy(ev[:, :Z], o[:, :Z])
                nc.sync.dma_start(outT_d[row0:row0 + 128, z0:z0 + Z],
                                  ev[:, :Z])

            tp0e = [p1[0], p1[1], p4]
            for m in range(2):
                emit_out(f"o0e_{m}", 128 * m,
                         [(L0e[ci][:, 128 * m:128 * (m + 1)], tp0e[ci])
                          for ci in range(3)])
            for j in range(3):
                tp1o = [p2[j][0], p2[j][1], p3[j]]
                emit_out(f"o1o_{j}", 256 + 128 * j,
                         [(L1o[ci][:, :], tp1o[ci]) for ci in range(3)])
            for j in range(3):
                emit_out(f"o1e_{j}", 640 + 128 * j, [(L1e[:, :], p5[j])])


def _prep_inputs(node_feat, w_00_0, w_01_1, w_10_1, w_11_0, w_11_1,
                 W_0e, W_1o, W_1e):
    weights = {
        "wt000": np.ascontiguousarray((C_000 * w_00_0).T, dtype=np.float32),
        "wt011": np.ascontiguousarray((C_011 * w_01_1).T, dtype=np.float32),
        "wt101": np.ascontiguousarray((C_101 * w_10_1).T, dtype=np.float32),
        "wt110": np.ascontiguousarray((C_110 * w_11_0).T, dtype=np.float32),
        "wt111": np.ascontiguousarray((C_111 * w_11_1).T, dtype=np.float32),
        "l0e": np.ascontiguousarray(W_0e / np.sqrt(384.0), dtype=np.float32),
        "l1o": np.ascontiguousarray(W_1o / np.sqrt(384.0), dtype=np.float32),
        "l1e": np.ascontiguousarray(W_1e / np.sqrt(128.0), dtype=np.float32),
    }
    feat = np.asarray(node_feat, dtype=np.float32).reshape(N_CORES, NS, 640)
    in_maps = []
    for i in range(N_CORES):
        blk = feat[i]
        xT = np.zeros((640, NSH), np.float32)
        xT[:256, :NS] = blk[:, :256].T
        vv = blk[:, 256:].reshape(NS, 128, 3)
        xT[256:, :NS] = vv.transpose(2, 1, 0).reshape(384, NS)
        in_maps.append({"xT": xT, **weights})
    return in_maps


def _gather(results):
    out = np.empty((N_NODES, 1024), np.float32)
    for i in range(N_CORES):
        oT = np.asarray(results[i]["outT"])[:, :NS]
        blk = out[i * NS:(i + 1) * NS]
        blk[:, :256] = oT[:256].T
        blk[:, 256:640] = oT[256:640].reshape(3, 128, NS).transpose(2, 1, 0) \
            .reshape(NS, 384)
        blk[:, 640:] = oT[640:].reshape(3, 128, NS).transpose(2, 1, 0) \
            .reshape(NS, 384)
    return out


def kernel(node_feat, w_00_0, w_01_1, w_10_1, w_11_0, w_11_1,
           W_0e, W_1o, W_1e, _trace=False):
    if "nc" not in _CACHE:
        _CACHE["nc"] = _build_program()
    nc = _CACHE["nc"]
    in_maps = _prep_inputs(node_feat, w_00_0, w_01_1, w_10_1, w_11_0,
                           w_11_1, W_0e, W_1o, W_1e)
    res = run_bass_kernel_spmd(nc, in_maps, core_ids=list(range(N_CORES)),
                               trace=_trace)
    out = _gather(res.results)
    if _trace:
        return out, res
    return out


# revision 21
# speedup vs baseline: 1.6938x; 1.6938x over previous
"""Trainium2 Bass kernel for nn_NodePreTrans (e3nn tensor product + linear).

Data-parallel over nodes: 50000 rows sharded 8 ways (6250/core, padded to
6272).  Channel-major on-device layout: all matmuls are (weights stationary)
[K,128] x [K,Z] with Z up to 512 nodes in the moving/free dimension.
"""

import sys

sys.path.insert(0, "/opt/trn_rl_repo")

import numpy as np

import concourse.bacc as bacc
import concourse.bass as bass
import concourse.mybir as mybir
import concourse.tile as tile
from concourse.bass_utils import run_bass_kernel_spmd

N_NODES = 50000
N_CORES = 8
NS = N_NODES // N_CORES          # 6250 real nodes per core
NSH = 6272                       # padded (12*512 + 128)
MUL_S = 256
MUL_V = 128

C_000 = 1.0 / np.sqrt(256.0)
C_011 = 1.0 / np.sqrt(128.0)
C_101 = 1.0 / np.sqrt(256.0)
C_110 = 1.0 / np.sqrt(384.0)
C_111 = 1.0 / 16.0

F32 = mybir.dt.float32
F32R = mybir.dt.float32r

Z_BLOCKS = [(i * 512, 512) for i in range(12)] + [(6144, 128)]

_CACHE = {}


def _build_program(variant="full"):
    nc = bacc.Bacc("TRN2", target_bir_lowering=False, debug=False,
                   num_devices=N_CORES)

    MDT = F32 if variant in ("dma", "mm") else F32R
    xT_d = nc.dram_tensor("xT", [640, NSH], MDT, kind="ExternalInput").ap()
    wt000_d = nc.dram_tensor("wt000", [256, 256], MDT, kind="ExternalInput").ap()
    wt011_d = nc.dram_tensor("wt011", [128, 256], MDT, kind="ExternalInput").ap()
    wt101_d = nc.dram_tensor("wt101", [256, 128], MDT, kind="ExternalInput").ap()
    wt110_d = nc.dram_tensor("wt110", [128, 128], MDT, kind="ExternalInput").ap()
    wt111_d = nc.dram_tensor("wt111", [128, 128], MDT, kind="ExternalInput").ap()
    l0e_d = nc.dram_tensor("l0e", [384, 256], MDT, kind="ExternalInput").ap()
    l1o_d = nc.dram_tensor("l1o", [384, 128], MDT, kind="ExternalInput").ap()
    l1e_d = nc.dram_tensor("l1e", [128, 128], MDT, kind="ExternalInput").ap()
    outT_d = nc.dram_tensor("outT", [1024, NSH], F32, kind="ExternalOutput").ap()

    with tile.TileContext(nc) as tc:
        _emit(tc, nc, xT_d, wt000_d, wt011_d, wt101_d, wt110_d, wt111_d,
              l0e_d, l1o_d, l1e_d, outT_d, variant=variant, mdt=MDT)

    nc.compile()
    return nc


def _emit(tc, nc, xT_d, wt000_d, wt011_d, wt101_d, wt110_d, wt111_d,
          l0e_d, l1o_d, l1e_d, outT_d, variant="full", mdt=F32R):
    with (
        tc.tile_pool(name="wpool", bufs=1) as wpool,
        tc.tile_pool(name="xin", bufs=2) as xin,
        tc.tile_pool(name="gat", bufs=2) as gat,
        tc.tile_pool(name="tmp", bufs=2) as tmp,
        tc.tile_pool(name="oev", bufs=2) as oev,
        tc.tile_pool(name="ps1", bufs=1, space="PSUM") as ps1,
        tc.tile_pool(name="ps2", bufs=1, space="PSUM") as ps2,
    ):
        # ---- resident weights -------------------------------------------
        def wtile(name, dram_ap, rows, cols):
            t = wpool.tile([128, cols], mdt, name=name)
            nc.sync.dma_start(t[:, :], dram_ap[rows:rows + 128, :])
            return t

        w000 = [wtile(f"w000_{k}", wt000_d, 128 * k, 256) for k in range(2)]
        w011 = wtile("w011", wt011_d, 0, 256)
        w101 = [wtile(f"w101_{k}", wt101_d, 128 * k, 128) for k in range(2)]
        w110 = wtile("w110", wt110_d, 0, 128)
        w111 = wtile("w111", wt111_d, 0, 128)
        L0e = [wtile(f"l0e_{k}", l0e_d, 128 * k, 256) for k in range(3)]
        L1o = [wtile(f"l1o_{k}", l1o_d, 128 * k, 128) for k in range(3)]
        L1e = wtile("l1e", l1e_d, 0, 128)

        for z0, Z in Z_BLOCKS:
            # ---- load x tiles (channel-major) ---------------------------
            s = []
            for m in range(2):
                t = xin.tile([128, 512], mdt, name=f"s{m}")
                nc.sync.dma_start(t[:, :Z], xT_d[128 * m:128 * (m + 1),
                                                 z0:z0 + Z])
                s.append(t)
            v = []
            for j in range(3):
                t = xin.tile([128, 512], mdt, name=f"v{j}")
                nc.sync.dma_start(t[:, :Z], xT_d[256 + 128 * j:384 + 128 * j,
                                                 z0:z0 + Z])
                v.append(t)

            def ps_tile():
                return ps1.tile([128, 512], F32, name="s1r", bufs=5)

            def mmr(out, lhsT, rhs, start, stop):
                nc.tensor.matmul(out, lhsT, rhs, start=start, stop=stop)

            if variant == "dma":
                for i, t in enumerate(s + v):
                    nc.sync.dma_start(outT_d[128 * i:128 * (i + 1),
                                             z0:z0 + Z], t[:, :Z])
                continue

            if variant == "mm":
                idx = 0
                for (w, rr) in [(w000[0], s[0]), (w000[1], s[1]),
                                (w011, v[0]), (w011, v[1]), (w011, v[2]),
                                (w101[0], s[0]), (w101[1], s[1]),
                                (w110, v[0]), (w110, v[1]), (w110, v[2]),
                                (w111, v[0]), (w111, v[1]), (w111, v[2])]:
                    a = ps_tile()
                    nc.tensor.matmul(a[:, :Z], w[:, :128], rr[:, :Z],
                                     start=True, stop=True)
                    ev = oev.tile([128, 512], F32, name=f"mmev{idx % 4}")
                    nc.scalar.copy(ev[:, :Z], a[:, :Z])
                    nc.sync.dma_start(outT_d[128 * (idx % 8):
                                             128 * (idx % 8) + 128,
                                             z0:z0 + Z], ev[:, :Z])
                    idx += 1
                continue

            # ---- path 1: p1 = s * (w00.T @ s) --------------------------
            p1 = []
            for m in range(2):
                a = ps_tile()
                mmr(a[:, :Z], w000[0][:, 128 * m:128 * (m + 1)],
                    s[0][:, :Z], start=True, stop=False)
                mmr(a[:, :Z], w000[1][:, 128 * m:128 * (m + 1)],
                    s[1][:, :Z], start=False, stop=True)
                p = gat.tile([128, 512], mdt, name=f"p1_{m}")
                nc.vector.tensor_mul(p[:, :Z], s[m][:, :Z], a[:, :Z])
                p1.append(p)

            # ---- path 2: p2_j = s * (w01.T @ v_j) ----------------------
            p2 = []
            for j in range(3):
                pj = []
                for m in range(2):
                    b = ps_tile()
                    mmr(b[:, :Z], w011[:, 128 * m:128 * (m + 1)],
                        v[j][:, :Z], start=True, stop=True)
                    p = gat.tile([128, 512], mdt, name=f"p2_{j}_{m}")
                    nc.vector.tensor_mul(p[:, :Z], s[m][:, :Z], b[:, :Z])
                    pj.append(p)
                p2.append(pj)

            # ---- path 3: p3_j = v_j * (w10.T @ s) ----------------------
            c = ps_tile()
            mmr(c[:, :Z], w101[0][:, :], s[0][:, :Z], start=True, stop=False)
            mmr(c[:, :Z], w101[1][:, :], s[1][:, :Z], start=False, stop=True)
            p3 = []
            for j in range(3):
                p = gat.tile([128, 512], mdt, name=f"p3_{j}")
                nc.vector.tensor_mul(p[:, :Z], v[j][:, :Z], c[:, :Z])
                p3.append(p)

            # ---- path 4: p4 = sum_j v_j * (w110.T @ v_j) ---------------
            p4 = gat.tile([128, 512], mdt, name="p4")
            for j in range(3):
                d = ps_tile()
                mmr(d[:, :Z], w110[:, :], v[j][:, :Z], start=True, stop=True)
                if j == 0:
                    nc.vector.tensor_mul(p4[:, :Z], v[0][:, :Z], d[:, :Z])
                else:
                    t4 = tmp.tile([128, 512], mdt, name="t4")
                    nc.vector.tensor_mul(t4[:, :Z], v[j][:, :Z], d[:, :Z])
                    nc.vector.tensor_add(p4[:, :Z], p4[:, :Z], t4[:, :Z])

            # ---- path 5: p5_k = v_i*E_j - v_j*E_i, (i,j)=(k+1,k+2)%3 ---
            # E is evacuated by ACT so the gating runs on GpSimd (SBUF-only),
            # offloading DVE which is otherwise the bottleneck engine.
            Ev = []
            for j in range(3):
                e = ps_tile()
                mmr(e[:, :Z], w111[:, :], v[j][:, :Z], start=True, stop=True)
                ev = gat.tile([128, 512], mdt, name=f"Ev{j}")
                nc.scalar.copy(ev[:, :Z], e[:, :Z])
                Ev.append(ev)
            p5 = []
            for k in range(3):
                i, j = (k + 1) % 3, (k + 2) % 3
                ta = tmp.tile([128, 512], mdt, name="t5a")
                tb = tmp.tile([128, 512], mdt, name="t5b")
                nc.gpsimd.tensor_mul(ta[:, :Z], v[i][:, :Z], Ev[j][:, :Z])
                nc.gpsimd.tensor_mul(tb[:, :Z], v[j][:, :Z], Ev[i][:, :Z])
                p = gat.tile([128, 512], mdt, name=f"p5_{k}")
                nc.gpsimd.tensor_sub(p[:, :Z], ta[:, :Z], tb[:, :Z])
                p5.append(p)

            if variant == "gat":
                outs8 = [p1[0], p1[1], p2[0][0], p2[0][1], p3[0], p4,
                         p5[0], p5[1]]
                for i, t in enumerate(outs8):
                    nc.sync.dma_start(outT_d[128 * i:128 * (i + 1),
                                             z0:z0 + Z], t[:, :Z])
                continue

            # ---- stage 2 linears + evacuate + store --------------------
            def emit_out(name, row0, chunks):
                o = ps2.tile([128, 512], F32, name="s2o", bufs=3)
                n = len(chunks)
                for ci, (lw, rhs) in enumerate(chunks):
                    mmr(o[:, :Z], lw, rhs[:, :Z],
                        start=(ci == 0), stop=(ci == n - 1))
                ev = oev.tile([128, 512], F32, name=name)
                nc.scalar.copy(ev[:, :Z], o[:, :Z])
                nc.sync.dma_start(outT_d[row0:row0 + 128, z0:z0 + Z],
                                  ev[:, :Z])

            tp0e = [p1[0], p1[1], p4]
            for m in range(2):
                emit_out(f"o0e_{m}", 128 * m,
                         [(L0e[ci][:, 128 * m:128 * (m + 1)], tp0e[ci])
                          for ci in range(3)])
            for j in range(3):
                tp1o = [p2[j][0], p2[j][1], p3[j]]
                emit_out(f"o1o_{j}", 256 + 128 * j,
                         [(L1o[ci][:, :], tp1o[ci]) for ci in range(3)])
            for j in range(3):
                emit_out(f"o1e_{j}", 640 + 128 * j, [(L1e[:, :], p5[j])])


def _prep_inputs(node_feat, w_00_0, w_01_1, w_10_1, w_11_0, w_11_1,
                 W_0e, W_1o, W_1e):
    weights = {
        "wt000": np.ascontiguousarray((C_000 * w_00_0).T, dtype=np.float32),
        "wt011": np.ascontiguousarray((C_011 * w_01_1).T, dtype=np.float32),
        "wt101": np.ascontiguousarray((C_101 * w_10_1).T, dtype=np.float32),
        "wt110": np.ascontiguousarray((C_110 * w_11_0).T, dtype=np.float32),
        "wt111": np.ascontiguousarray((C_111 * w_11_1).T, dtype=np.float32),
        "l0e": np.ascontiguousarray(W_0e / np.sqrt(384.0), dtype=np.float32),
        "l1o": np.ascontiguousarray(W_1o / np.sqrt(384.0), dtype=np.float32),
        "l1e": np.ascontiguousarray(W_1e / np.sqrt(128.0), dtype=np.float32),
    }
    feat = np.asarray(node_feat, dtype=np.float32).reshape(N_CORES, NS, 640)
    in_maps = []
    for i in range(N_CORES):
        blk = feat[i]
        xT = np.zeros((640, NSH), np.float32)
        xT[:256, :NS] = blk[:, :256].T
        vv = blk[:, 256:].reshape(NS, 128, 3)
        xT[256:, :NS] = vv.transpose(2, 1, 0).reshape(384, NS)
        in_maps.append({"xT": xT, **weights})
    return in_maps


def _gather(results):
    out = np.empty((N_NODES, 1024), np.float32)
    for i in range(N_CORES):
        oT = np.asarray(results[i]["outT"])[:, :NS]
        blk = out[i * NS:(i + 1) * NS]
        blk[:, :256] = oT[:256].T
        blk[:, 256:640] = oT[256:640].reshape(3, 128, NS).transpose(2, 1, 0) \
            .reshape(NS, 384)
        blk[:, 640:] = oT[640:].reshape(3, 128, NS).transpose(2, 1, 0) \
            .reshape(NS, 384)
    return out


def kernel(node_feat, w_00_0, w_01_1, w_10_1, w_11_0, w_11_1,
           W_0e, W_1o, W_1e, _trace=False):
    if "nc" not in _CACHE:
        _CACHE["nc"] = _build_program()
    nc = _CACHE["nc"]
    in_maps = _prep_inputs(node_feat, w_00_0, w_01_1, w_10_1, w_11_0,
                           w_11_1, W_0e, W_1o, W_1e)
    res = run_bass_kernel_spmd(nc, in_maps, core_ids=list(range(N_CORES)),
                               trace=_trace)
    out = _gather(res.results)
    if _trace:
        return out, res
    return out


# revision 23
# speedup vs baseline: 2.5294x; 1.4933x over previous
"""Trainium2 Bass kernel for nn_NodePreTrans (e3nn tensor product + linear).

Data-parallel over nodes: 50000 rows sharded 8 ways (6250/core, padded to
6272).  Channel-major on-device layout: all matmuls are (weights stationary)
[K,128] x [K,Z] with Z up to 512 nodes in the moving/free dimension.
"""

import sys

sys.path.insert(0, "/opt/trn_rl_repo")

import numpy as np

import concourse.bacc as bacc
import concourse.bass as bass
import concourse.mybir as mybir
import concourse.tile as tile
from concourse.bass_utils import run_bass_kernel_spmd

N_NODES = 50000
N_CORES = 8
NS = N_NODES // N_CORES          # 6250 real nodes per core
NSH = 6272                       # padded (12*512 + 128)
MUL_S = 256
MUL_V = 128

C_000 = 1.0 / np.sqrt(256.0)
C_011 = 1.0 / np.sqrt(128.0)
C_101 = 1.0 / np.sqrt(256.0)
C_110 = 1.0 / np.sqrt(384.0)
C_111 = 1.0 / 16.0

F32 = mybir.dt.float32
F32R = mybir.dt.float32r

Z_BLOCKS = [(i * 512, 512) for i in range(12)] + [(6144, 128)]

_CACHE = {}


def _build_program(variant="full"):
    nc = bacc.Bacc("TRN2", target_bir_lowering=False, debug=False,
                   num_devices=N_CORES)

    MDT = F32 if variant in ("dma", "mm") else F32R
    xT_d = nc.dram_tensor("xT", [640, NSH], MDT, kind="ExternalInput").ap()
    wt000_d = nc.dram_tensor("wt000", [256, 256], MDT, kind="ExternalInput").ap()
    wt011_d = nc.dram_tensor("wt011", [128, 256], MDT, kind="ExternalInput").ap()
    wt101_d = nc.dram_tensor("wt101", [256, 128], MDT, kind="ExternalInput").ap()
    wt110_d = nc.dram_tensor("wt110", [128, 128], MDT, kind="ExternalInput").ap()
    wt111_d = nc.dram_tensor("wt111", [128, 128], MDT, kind="ExternalInput").ap()
    l0e_d = nc.dram_tensor("l0e", [384, 256], MDT, kind="ExternalInput").ap()
    l1o_d = nc.dram_tensor("l1o", [384, 128], MDT, kind="ExternalInput").ap()
    l1e_d = nc.dram_tensor("l1e", [128, 128], MDT, kind="ExternalInput").ap()
    outT_d = nc.dram_tensor("outT", [1024, NSH], F32, kind="ExternalOutput").ap()

    with tile.TileContext(nc) as tc:
        _emit(tc, nc, xT_d, wt000_d, wt011_d, wt101_d, wt110_d, wt111_d,
              l0e_d, l1o_d, l1e_d, outT_d, variant=variant, mdt=MDT)

    nc.compile()
    return nc


def _emit(tc, nc, xT_d, wt000_d, wt011_d, wt101_d, wt110_d, wt111_d,
          l0e_d, l1o_d, l1e_d, outT_d, variant="full", mdt=F32R):
    with (
        tc.tile_pool(name="wpool", bufs=1) as wpool,
        tc.tile_pool(name="xin", bufs=2) as xin,
        tc.tile_pool(name="gat", bufs=2) as gat,
        tc.tile_pool(name="tmp", bufs=4) as tmp,
        tc.tile_pool(name="oev", bufs=2) as oev,
        tc.tile_pool(name="ps1", bufs=1, space="PSUM") as ps1,
        tc.tile_pool(name="ps2", bufs=1, space="PSUM") as ps2,
    ):
        # ---- resident weights -------------------------------------------
        def wtile(name, dram_ap, rows, cols):
            t = wpool.tile([128, cols], mdt, name=name)
            nc.sync.dma_start(t[:, :], dram_ap[rows:rows + 128, :])
            return t

        w000 = [wtile(f"w000_{k}", wt000_d, 128 * k, 256) for k in range(2)]
        w011 = wtile("w011", wt011_d, 0, 256)
        w101 = [wtile(f"w101_{k}", wt101_d, 128 * k, 128) for k in range(2)]
        w110 = wtile("w110", wt110_d, 0, 128)
        w111 = wtile("w111", wt111_d, 0, 128)
        L0e = [wtile(f"l0e_{k}", l0e_d, 128 * k, 256) for k in range(3)]
        L1o = [wtile(f"l1o_{k}", l1o_d, 128 * k, 128) for k in range(3)]
        L1e = wtile("l1e", l1e_d, 0, 128)

        for z0, Z in Z_BLOCKS:
            # ---- load x tiles (channel-major) ---------------------------
            s = []
            for m in range(2):
                t = xin.tile([128, 512], mdt, name=f"s{m}")
                nc.sync.dma_start(t[:, :Z], xT_d[128 * m:128 * (m + 1),
                                                 z0:z0 + Z])
                s.append(t)
            v = []
            for j in range(3):
                t = xin.tile([128, 512], mdt, name=f"v{j}")
                nc.sync.dma_start(t[:, :Z], xT_d[256 + 128 * j:384 + 128 * j,
                                                 z0:z0 + Z])
                v.append(t)

            def ps_tile():
                return ps1.tile([128, 512], F32, name="s1r", bufs=5)

            def mmr(out, lhsT, rhs, start, stop):
                nc.tensor.matmul(out, lhsT, rhs, start=start, stop=stop)

            if variant == "dma":
                for i, t in enumerate(s + v):
                    nc.sync.dma_start(outT_d[128 * i:128 * (i + 1),
                                             z0:z0 + Z], t[:, :Z])
                continue

            if variant == "mm":
                idx = 0
                for (w, rr) in [(w000[0], s[0]), (w000[1], s[1]),
                                (w011, v[0]), (w011, v[1]), (w011, v[2]),
                                (w101[0], s[0]), (w101[1], s[1]),
                                (w110, v[0]), (w110, v[1]), (w110, v[2]),
                                (w111, v[0]), (w111, v[1]), (w111, v[2])]:
                    a = ps_tile()
                    nc.tensor.matmul(a[:, :Z], w[:, :128], rr[:, :Z],
                                     start=True, stop=True)
                    ev = oev.tile([128, 512], F32, name=f"mmev{idx % 4}")
                    nc.scalar.copy(ev[:, :Z], a[:, :Z])
                    nc.sync.dma_start(outT_d[128 * (idx % 8):
                                             128 * (idx % 8) + 128,
                                             z0:z0 + Z], ev[:, :Z])
                    idx += 1
                continue

            # ---- path 1: p1 = s * (w00.T @ s) --------------------------
            p1 = []
            for m in range(2):
                a = ps_tile()
                mmr(a[:, :Z], w000[0][:, 128 * m:128 * (m + 1)],
                    s[0][:, :Z], start=True, stop=False)
                mmr(a[:, :Z], w000[1][:, 128 * m:128 * (m + 1)],
                    s[1][:, :Z], start=False, stop=True)
                p = gat.tile([128, 512], mdt, name=f"p1_{m}")
                nc.vector.tensor_mul(p[:, :Z], s[m][:, :Z], a[:, :Z])
                p1.append(p)

            # ---- path 2: p2_j = s * (w01.T @ v_j) ----------------------
            p2 = []
            for j in range(3):
                pj = []
                for m in range(2):
                    b = ps_tile()
                    mmr(b[:, :Z], w011[:, 128 * m:128 * (m + 1)],
                        v[j][:, :Z], start=True, stop=True)
                    p = gat.tile([128, 512], mdt, name=f"p2_{j}_{m}")
                    nc.vector.tensor_mul(p[:, :Z], s[m][:, :Z], b[:, :Z])
                    pj.append(p)
                p2.append(pj)

            # ---- path 3: p3_j = v_j * (w10.T @ s) ----------------------
            c = ps_tile()
            mmr(c[:, :Z], w101[0][:, :], s[0][:, :Z], start=True, stop=False)
            mmr(c[:, :Z], w101[1][:, :], s[1][:, :Z], start=False, stop=True)
            p3 = []
            for j in range(3):
                p = gat.tile([128, 512], mdt, name=f"p3_{j}")
                nc.vector.tensor_mul(p[:, :Z], v[j][:, :Z], c[:, :Z])
                p3.append(p)

            # ---- path 4: p4 = sum_j v_j * (w110.T @ v_j) ---------------
            # muls on DVE (read PSUM); accumulate adds on GpSimd (SBUF-only)
            p4 = gat.tile([128, 512], mdt, name="p4")
            for j in range(3):
                d = ps_tile()
                mmr(d[:, :Z], w110[:, :], v[j][:, :Z], start=True, stop=True)
                if j == 0:
                    nc.vector.tensor_mul(p4[:, :Z], v[0][:, :Z], d[:, :Z])
                else:
                    t4 = tmp.tile([128, 512], mdt, name="t4")
                    nc.vector.tensor_mul(t4[:, :Z], v[j][:, :Z], d[:, :Z])
                    nc.gpsimd.tensor_add(p4[:, :Z], p4[:, :Z], t4[:, :Z])

            # ---- path 5: p5_k = v_i*E_j - v_j*E_i, (i,j)=(k+1,k+2)%3 ---
            # muls on DVE (read E from PSUM); final subs on GpSimd (SBUF-only)
            E = []
            for j in range(3):
                e = ps_tile()
                mmr(e[:, :Z], w111[:, :], v[j][:, :Z], start=True, stop=True)
                E.append(e)
            p5 = []
            for k in range(3):
                i, j = (k + 1) % 3, (k + 2) % 3
                ta = tmp.tile([128, 512], mdt, name="t5a")
                tb = tmp.tile([128, 512], mdt, name="t5b")
                nc.vector.tensor_mul(ta[:, :Z], v[i][:, :Z], E[j][:, :Z])
                nc.vector.tensor_mul(tb[:, :Z], v[j][:, :Z], E[i][:, :Z])
                p = gat.tile([128, 512], mdt, name=f"p5_{k}")
                nc.gpsimd.tensor_sub(p[:, :Z], ta[:, :Z], tb[:, :Z])
                p5.append(p)

            if variant == "gat":
                outs8 = [p1[0], p1[1], p2[0][0], p2[0][1], p3[0], p4,
                         p5[0], p5[1]]
                for i, t in enumerate(outs8):
                    nc.sync.dma_start(outT_d[128 * i:128 * (i + 1),
                                             z0:z0 + Z], t[:, :Z])
                continue

            # ---- stage 2 linears + evacuate + store --------------------
            def emit_out(name, row0, chunks):
                o = ps2.tile([128, 512], F32, name="s2o", bufs=3)
                n = len(chunks)
                for ci, (lw, rhs) in enumerate(chunks):
                    mmr(o[:, :Z], lw, rhs[:, :Z],
                        start=(ci == 0), stop=(ci == n - 1))
                ev = oev.tile([128, 512], F32, name=name)
                nc.scalar.copy(ev[:, :Z], o[:, :Z])
                nc.sync.dma_start(outT_d[row0:row0 + 128, z0:z0 + Z],
                                  ev[:, :Z])

            tp0e = [p1[0], p1[1], p4]
            for m in range(2):
                emit_out(f"o0e_{m}", 128 * m,
                         [(L0e[ci][:, 128 * m:128 * (m + 1)], tp0e[ci])
                          for ci in range(3)])
            for j in range(3):
                tp1o = [p2[j][0], p2[j][1], p3[j]]
                emit_out(f"o1o_{j}", 256 + 128 * j,
                         [(L1o[ci][:, :], tp1o[ci]) for ci in range(3)])
            for j in range(3):
                emit_out(f"o1e_{j}", 640 + 128 * j, [(L1e[:, :], p5[j])])


def _prep_inputs(node_feat, w_00_0, w_01_1, w_10_1, w_11_0, w_11_1,
                 W_0e, W_1o, W_1e):
    weights = {
        "wt000": np.ascontiguousarray((C_000 * w_00_0).T, dtype=np.float32),
        "wt011": np.ascontiguousarray((C_011 * w_01_1).T, dtype=np.float32),
        "wt101": np.ascontiguousarray((C_101 * w_10_1).T, dtype=np.float32),
        "wt110": np.ascontiguousarray((C_110 * w_11_0).T, dtype=np.float32),
        "wt111": np.ascontiguousarray((C_111 * w_11_1).T, dtype=np.float32),
        "l0e": np.ascontiguousarray(W_0e / np.sqrt(384.0), dtype=np.float32),
        "l1o": np.ascontiguousarray(W_1o / np.sqrt(384.0), dtype=np.float32),
        "l1e": np.ascontiguousarray(W_1e / np.sqrt(128.0), dtype=np.float32),
    }
    feat = np.asarray(node_feat, dtype=np.float32).reshape(N_CORES, NS, 640)
    in_maps = []
    for i in range(N_CORES):
        blk = feat[i]
        xT = np.zeros((640, NSH), np.float32)
        xT[:256, :NS] = blk[:, :256].T
        vv = blk[:, 256:].reshape(NS, 128, 3)
        xT[256:, :NS] = vv.transpose(2, 1, 0).reshape(384, NS)
        in_maps.append({"xT": xT, **weights})
    return in_maps


def _gather(results):
    out = np.empty((N_NODES, 1024), np.float32)
    for i in range(N_CORES):
        oT = np.asarray(results[i]["outT"])[:, :NS]
        blk = out[i * NS:(i + 1) * NS]
        blk[:, :256] = oT[:256].T
        blk[:, 256:640] = oT[256:640].reshape(3, 128, NS).transpose(2, 1, 0) \
            .reshape(NS, 384)
        blk[:, 640:] = oT[640:].reshape(3, 128, NS).transpose(2, 1, 0) \
            .reshape(NS, 384)
    return out


def kernel(node_feat, w_00_0, w_01_1, w_10_1, w_11_0, w_11_1,
           W_0e, W_1o, W_1e, _trace=False):
    if "nc" not in _CACHE:
        _CACHE["nc"] = _build_program()
    nc = _CACHE["nc"]
    in_maps = _prep_inputs(node_feat, w_00_0, w_01_1, w_10_1, w_11_0,
                           w_11_1, W_0e, W_1o, W_1e)
    res = run_bass_kernel_spmd(nc, in_maps, core_ids=list(range(N_CORES)),
                               trace=_trace)
    out = _gather(res.results)
    if _trace:
        return out, res
    return out
